# revision 13
# baseline (speedup 1.0000x reference)
"""BiLSTM Trainium2 kernel.

Strategy: 8 NeuronCores = 2 directions x 4 batch-groups (8 sequences each).
No cross-core communication: each core runs the full recurrence for its
(direction, batch-group) with that direction's weights resident in SBUF.
Backward direction is realized by feeding time-reversed inputs (the program
is identical on every core; only per-core input data differs).

Per core:
  phase 1: x_h[ns, j] = x_part @ W_ih_dir  (bf16 matmuls, fp32 accumulate)
           stored to a DRAM scratch buffer.
  phase 2: 256 sequential LSTM steps. Gate matmul h_{t-1} @ W_hh_dir uses
           4-way PE column-tiling (one 32-partition group per gate) so all
           four gates' weight columns stream concurrently; cell math in fp32
           on DVE/ACT; h is PE-transposed each step to form the next
           stationary operand.

Host assembles hidden_states/final_state from the 8 cores' outputs.
"""

import os
import numpy as np

N, S, D, H = 32, 256, 1024, 1024
NB = 8          # batch per core
KC = D // 128   # contraction chunks
GJ = 4 * H      # gate-major column count per direction

_CACHE = {}


def _build(nc_mod, S_steps):
    import concourse.bass as bass
    import concourse.bacc as bacc
    import concourse.mybir as mybir

    dt = mybir.dt
    NSC = NB * S_steps // 128      # phase-1 ns chunks
    assert NB * S_steps % 128 == 0

    nc = bacc.Bacc(None, num_devices=1)

    # ---- DRAM parameters (per-core data; same names on every core) ----
    xt = nc.declare_dram_parameter("xt", [128, KC * NSC * 128], dt.bfloat16, isOutput=False)
    wih = nc.declare_dram_parameter("wih", [128, KC * GJ], dt.bfloat16, isOutput=False)
    whh = nc.declare_dram_parameter("whh", [128, KC * GJ], dt.bfloat16, isOutput=False)
    ident = nc.declare_dram_parameter("ident", [128, 8], dt.float32, isOutput=False)
    out_h = nc.declare_dram_parameter("out_h", [S_steps, NB, H], dt.float32, isOutput=True)
    xh_dram = nc.dram_tensor("xh_dram", [NB * S_steps, GJ], dt.float32)

    from contextlib import ExitStack
    with ExitStack() as _ctx:
        def _sb(name, shape, dtype):
            return _ctx.enter_context(nc.sbuf_tensor(name, shape, dtype))
        def _sem(name):
            return _ctx.enter_context(nc.semaphore(name))
        wih_sb = _sb("wih_sb", [128, KC * GJ], dt.bfloat16)
        whh_sb = _sb("whh_sb", [128, KC * GJ], dt.bfloat16)
        xt_ring = _sb("xt_ring", [128, 16 * 128], dt.bfloat16)
        xh_stage = _sb("xh_stage", [128, GJ], dt.float32)
        ident_sb = _sb("ident_sb", [128, 8], dt.float32)
        xh_ring = _sb("xh_ring", [128, 4 * H], dt.float32)
        pre_sb = _sb("pre_sb", [128, H], dt.float32)
        gates_col = _sb("gates_col", [8, 4 * H], dt.float32)
        cell_col = _sb("cell_col", [8, 4 * H], dt.float32)
        h_pack = _sb("h_pack", [128, H], dt.float32)
        sig_f = gates_col[:, 0:H]
        sig_i = gates_col[:, H:2 * H]
        sig_o = gates_col[:, 2 * H:3 * H]
        tan_g = gates_col[:, 3 * H:4 * H]
        c_sb = cell_col[:, 0:H]
        c_tmp = cell_col[:, H:2 * H]
        ig_tmp = cell_col[:, 2 * H:3 * H]
        tanhc_sb = cell_col[:, 3 * H:4 * H]
        hT_bf = _sb("hT_bf", [128, 2 * 64], dt.bfloat16)
        s_wih = _sem("s_wih")
        s_whh = _sem("s_whh")
        s_xt = _sem("s_xt")
        s_p1mm = _sem("s_p1mm")
        s_p1cp = _sem("s_p1cp")
        s_xhout = _sem("s_xhout")
        s_xh = [_sem(f"s_xh{k}") for k in range(4)]
        s_mm = _sem("s_mm")
        s_pre = _sem("s_pre")
        s_sig = _sem("s_sig")
        s_c = _sem("s_c")
        s_tanhc = _sem("s_tanhc")
        s_h = _sem("s_h")
        s_tp = _sem("s_tp")
        s_hT = _sem("s_hT")
        s_hop = [_sem("s_hop0"), _sem("s_hop1")]

        AF = mybir.ActivationFunctionType

        XH_PITCH = 4 * H



        def xh_ring_gate(slot, g):
            return xh_ring[32 * g:32 * g + 8, slot * H:(slot + 1) * H]

        def xh_dram_gate(u, g):
            # [8n, H] tile from DRAM: rows n*S+u, cols g*H + j
            return xh_dram.rearrange("(n t) (g j) -> t g n j", n=NB, g=4)[u, g]

        def pre_ap(g):
            return pre_sb[32 * g:32 * g + 8, :]

        # ================= Block 1: loads + phase 1 =================
        with (
            nc.psum_tensor("ps1a", [128, 2048], dt.float32) as ps1a,
            nc.psum_tensor("ps1b", [128, 2048], dt.float32) as ps1b,
            nc.Block() as b1,
        ):
            ps1 = [ps1a, ps1b]

            @b1.sync
            def _(sy):
                for kcc in range(KC):
                    sy.dma_start(wih_sb[:, kcc * GJ:(kcc + 1) * GJ],
                                 wih[:, kcc * GJ:(kcc + 1) * GJ]).then_inc(s_wih, 16)
                sy.dma_start(ident_sb[:], ident[:]).then_inc(s_wih, 16)
                def store_xh(m):
                    sy.wait_ge(s_p1cp, m * 2 + 2)
                    sy.dma_start(
                        xh_dram[m * 128:(m + 1) * 128, :],
                        xh_stage[:, :],
                    ).then_inc(s_xhout, 16)

                for nsc in range(NSC):
                    if nsc >= 2:
                        store_xh(nsc - 2)
                    if nsc >= 1:
                        # ring slot reuse: PE matmuls of nsc-1 retired
                        # (proven by the DVE copies that follow them)
                        sy.wait_ge(s_p1cp, 2 * nsc)
                    for kcc in range(KC):
                        sy.dma_start(
                            xt_ring[:, kcc * 128:(kcc + 1) * 128],
                            xt[:, (kcc * NSC + nsc) * 128:(kcc * NSC + nsc) * 128 + 128],
                        ).then_inc(s_xt, 16)
                    # spread whh loads through phase 1
                    if nsc % 2 == 1 and nsc // 2 < KC:
                        kcc = nsc // 2
                        sy.dma_start(whh_sb[:, kcc * GJ:(kcc + 1) * GJ],
                                     whh[:, kcc * GJ:(kcc + 1) * GJ]).then_inc(s_whh, 16)
                for kcc in range(max(0, NSC // 2), KC):
                    sy.dma_start(whh_sb[:, kcc * GJ:(kcc + 1) * GJ],
                                 whh[:, kcc * GJ:(kcc + 1) * GJ]).then_inc(s_whh, 16)
                for m in range(max(0, NSC - 2), NSC):
                    store_xh(m)
                sy.wait_ge(s_xhout, 16 * NSC)

            @b1.tensor
            def _(te):
                te.wait_ge(s_wih, 16 * (KC + 1))  # wih + ident
                for nsc in range(NSC):
                    te.wait_ge(s_xt, 16 * KC * (nsc + 1))
                    for jh in range(2):
                        if nsc > 0:
                            te.wait_ge(s_p1cp, (nsc - 1) * 2 + jh + 1)
                        for kcc in range(KC):
                            slot = kcc
                            for jc in range(4):
                                j0 = jh * 2048 + jc * 512
                                ins = nc.tensor.matmul(
                                    ps1[jh][:, jc * 512:(jc + 1) * 512],
                                    xt_ring[:, slot * 128:(slot + 1) * 128],
                                    wih_sb[:, kcc * GJ + j0: kcc * GJ + j0 + 512],
                                    start=(kcc == 0), stop=(kcc == KC - 1),
                                )
                                if kcc == KC - 1 and jc == 3:
                                    ins.then_inc(s_p1mm, 1)

            @b1.vector
            def _(ve):
                for nsc in range(NSC):
                    for jh in range(2):
                        ve.wait_ge(s_p1mm, nsc * 2 + jh + 1)
                        if nsc >= 1:
                            ve.wait_ge(s_xhout, 16 * nsc)
                        for jc in range(4):
                            ins = nc.vector.tensor_copy(
                                xh_stage[:, jh * 2048 + jc * 512:
                                         jh * 2048 + (jc + 1) * 512],
                                ps1[jh][:, jc * 512:(jc + 1) * 512],
                            )
                        ins.then_inc(s_p1cp, 1)

        # ================= Block 2: recurrence =================
        with (
            nc.psum_tensor("psg0", [128, H], dt.float32) as psg0,
            nc.psum_tensor("psg1", [128, H], dt.float32) as psg1,
            nc.psum_tensor("pst0", [128, 64], dt.float32) as pst0,
            nc.psum_tensor("pst1", [128, 64], dt.float32) as pst1,
            nc.Block() as b2,
        ):
            ps_G = [psg0, psg1]
            ps_T = [pst0, pst1]
            PF = 3

            @b2.sync
            def _(sy):
                for u in range(min(PF, S_steps)):
                    for g in range(4):
                        sy.dma_start(xh_ring_gate(u % 4, g),
                                     xh_dram_gate(u, g)).then_inc(s_xh[u % 4], 16)
                for u in range(S_steps):
                    if u + PF < S_steps:
                        # slot (u+PF)%4 free when pre-adds of step u+PF-4 done
                        if u + PF - 4 >= 0:
                            sy.wait_ge(s_pre, u + PF - 3)
                        for g in range(4):
                            sy.dma_start(xh_ring_gate((u + PF) % 4, g),
                                         xh_dram_gate(u + PF, g)).then_inc(s_xh[(u + PF) % 4], 16)
                    # store h(u)
                    sy.wait_ge(s_h, u + 1)
                    sy.dma_start(out_h[u], h_pack[32 * (u % 2):32 * (u % 2) + 8, :]
                                 ).then_inc(s_hop[u % 2], 16)
                sy.wait_ge(s_hop[(S_steps - 1) % 2], 16 * ((S_steps + 1) // 2))
                sy.wait_ge(s_hop[(S_steps - 2) % 2], 16 * (S_steps // 2))

            @b2.tensor
            def _(te):
                te.wait_ge(s_whh, 16 * KC)  # whh resident
                for u in range(1, S_steps):
                    # transposes of h(u-1)
                    te.wait_ge(s_h, u)
                    if u >= 3:
                        te.wait_ge(s_hT, u - 1)  # ps_T[(u-1)%2] drained
                    for kcc in range(KC):
                        ins = nc.tensor.transpose(
                            ps_T[(u - 1) % 2][:, kcc * 8:(kcc + 1) * 8],
                            h_pack[32 * ((u - 1) % 2):32 * ((u - 1) % 2) + 8,
                                   kcc * 128:(kcc + 1) * 128],
                            ident_sb[32 * ((u - 1) % 2):32 * ((u - 1) % 2) + 8, :],
                        )
                    ins.then_inc(s_tp, 1)  # s_tp == u
                    # gate matmuls of step u (needs hT of u-1)
                    te.wait_ge(s_hT, u)
                    if u >= 2:
                        te.wait_ge(s_pre, u - 1)  # ps_G[u%2] consumed by step u-2
                    for kcc in range(KC):
                        for g in range(4):
                            for jc in range(2):
                                ins = nc.tensor.matmul(
                                    ps_G[u % 2][32 * g:32 * g + 8, jc * 512:(jc + 1) * 512],
                                    hT_bf[:, ((u - 1) % 2) * 64 + kcc * 8:
                                          ((u - 1) % 2) * 64 + (kcc + 1) * 8],
                                    whh_sb[:, kcc * GJ + g * H + jc * 512:
                                           kcc * GJ + g * H + (jc + 1) * 512],
                                    start=(kcc == 0), stop=(kcc == KC - 1),
                                    tile_position=(0, 32 * g),
                                )
                    ins.then_inc(s_mm, 1)  # s_mm == u

            @b2.vector
            def _(ve):
                for u in range(S_steps):
                    if u >= 1:
                        # copy transposed h(u-1) -> bf16 stationary
                        ve.wait_ge(s_tp, u)
                        nc.vector.tensor_copy(
                            hT_bf[:, ((u - 1) % 2) * 64:((u - 1) % 2) * 64 + 64],
                            ps_T[(u - 1) % 2][:, 0:64],
                        ).then_inc(s_hT, 1)  # s_hT == u
                    # pre-activations
                    ve.wait_ge(s_xh[u % 4], 64 * (u // 4 + 1))
                    if u >= 1:
                        ve.wait_ge(s_mm, u)
                    if u >= 1:
                        ve.wait_ge(s_sig, u)  # pre_sb free (acts of u-1 done)
                    for g in range(4):
                        if u == 0:
                            ins = nc.vector.tensor_copy(pre_ap(g), xh_ring_gate(u % 4, g))
                        else:
                            ins = nc.vector.tensor_add(
                                pre_ap(g), ps_G[u % 2][32 * g:32 * g + 8, :],
                                xh_ring_gate(u % 4, g))
                    ins.then_inc(s_pre, 1)  # s_pre == u+1
                    # cell update
                    ve.wait_ge(s_sig, u + 1)
                    nc.vector.tensor_mul(ig_tmp, sig_i, tan_g)
                    if u >= 1:
                        ve.wait_ge(s_tanhc, u)  # c_sb read by tanh of u-1
                        nc.vector.tensor_mul(c_tmp, sig_f, c_sb)
                        ve.drain()
                        nc.vector.tensor_add(c_sb, c_tmp, ig_tmp).then_inc(s_c, 1)
                    else:
                        ve.drain()
                        nc.vector.tensor_copy(c_sb, ig_tmp).then_inc(s_c, 1)
                    # h = o * tanh(c)
                    ve.wait_ge(s_tanhc, u + 1)
                    if u >= 2:
                        ve.wait_ge(s_tp, u - 1)      # transposes of h(u-2) done
                        ve.wait_ge(s_hop[u % 2], 16 * (u // 2))  # out of h(u-2) done
                    nc.vector.tensor_mul(
                        h_pack[32 * (u % 2):32 * (u % 2) + 8, :],
                        sig_o, tanhc_sb,
                    ).then_inc(s_h, 1)  # s_h == u+1

            @b2.scalar
            def _(ac):
                for u in range(S_steps):
                    ac.wait_ge(s_pre, u + 1)
                    if u >= 1:
                        ac.wait_ge(s_h, u)  # sig tiles read by h/cell of u-1
                    nc.scalar.activation(sig_f, pre_ap(0), AF.Sigmoid)
                    nc.scalar.activation(sig_i, pre_ap(1), AF.Sigmoid)
                    nc.scalar.activation(sig_o, pre_ap(2), AF.Sigmoid)
                    nc.scalar.activation(tan_g, pre_ap(3), AF.Tanh).then_inc(s_sig, 1)
                    ac.wait_ge(s_c, u + 1)
                    nc.scalar.activation(tanhc_sb, c_sb, AF.Tanh).then_inc(s_tanhc, 1)

    nc.finalize()
    return nc


def _get_program(S_steps):
    key = S_steps
    if key not in _CACHE:
        _CACHE[key] = _build(None, S_steps)
    return _CACHE[key]


def _prep_core_inputs(x, weight_ih, weight_hh, S_steps):
    """Build the 8 per-core input dicts."""
    import ml_dtypes
    bf16 = ml_dtypes.bfloat16
    NSC = NB * S_steps // 128
    ident = np.zeros((128, 8), np.float32)
    ident[0:8, :] = np.eye(8, dtype=np.float32)
    ident[32:40, :] = np.eye(8, dtype=np.float32)
    in_maps = []
    for c in range(8):
        d = c // 4          # 0 fwd, 1 bwd
        bg = c % 4
        xs = x[bg * NB:(bg + 1) * NB, :S_steps]
        if d == 1:
            xs = xs[:, ::-1]
        # xt: [128, KC*NSC*128], tile (kc, nsc) at cols (kc*NSC+nsc)*128
        xT = np.ascontiguousarray(xs.reshape(NB * S_steps, D).T)        # [D, ns]
        xt_t = xT.reshape(KC, 128, NSC, 128).transpose(1, 0, 2, 3).reshape(128, KC * NSC * 128)
        gih = weight_ih[d * 4:(d + 1) * 4]     # [4, D, H]
        ghh = weight_hh[d * 4:(d + 1) * 4]
        wih_arr = np.concatenate([gih[g] for g in range(4)], axis=1)    # [D, 4H]
        whh_arr = np.concatenate([ghh[g] for g in range(4)], axis=1)
        wih_t = wih_arr.reshape(KC, 128, GJ).transpose(1, 0, 2).reshape(128, KC * GJ)
        whh_t = whh_arr.reshape(KC, 128, GJ).transpose(1, 0, 2).reshape(128, KC * GJ)
        in_maps.append({
            "xt": xt_t.astype(bf16),
            "wih": wih_t.astype(bf16),
            "whh": whh_t.astype(bf16),
            "ident": ident,
        })
    return in_maps


def _run_device(x, weight_ih, weight_hh, S_steps):
    from concourse.bass_utils import run_bass_kernel_spmd
    nc = _get_program(S_steps)
    in_maps = _prep_core_inputs(x, weight_ih, weight_hh, S_steps)
    res = run_bass_kernel_spmd(nc, in_maps, list(range(8)))
    # assemble
    hs_f = np.empty((N, S_steps, H), np.float32)
    hs_b = np.empty((N, S_steps, H), np.float32)
    for c in range(8):
        d = c // 4
        bg = c % 4
        oh = res.results[c]["out_h"]        # [S, NB, H]
        if d == 0:
            hs_f[bg * NB:(bg + 1) * NB] = oh.transpose(1, 0, 2)
        else:
            hs_b[bg * NB:(bg + 1) * NB] = oh[::-1].transpose(1, 0, 2)
    return hs_f, hs_b


def _numpy_ref(x, mask, weight_ih, weight_hh, bias):
    """Exact fp32 fallback (used only if mask/bias are nonstandard)."""
    n, s, d = x.shape
    h = weight_hh.shape[-1]
    x_h = np.einsum("nsd,gdh->gnsh", x, weight_ih) + bias[:, None]
    hs_f = np.zeros((n, s, h), np.float32)
    hs_b = np.zeros((n, s, h), np.float32)

    def sigmoid(v):
        return 1.0 / (1.0 + np.exp(-v))

    for d_i, (sl, rng) in enumerate(((slice(0, 4), range(s)),
                                     (slice(4, 8), range(s - 1, -1, -1)))):
        W = weight_hh[sl]
        xh = x_h[sl]
        hid = np.zeros((n, h), np.float32)
        c = np.zeros((n, h), np.float32)
        for t in rng:
            pre = xh[:, :, t] + np.einsum("nh,ghk->gnk", hid, W)
            f = sigmoid(pre[0]); i = sigmoid(pre[1]); o = sigmoid(pre[2])
            c = f * c + i * np.tanh(pre[3])
            h_new = o * np.tanh(c)
            m = mask[:, t:t + 1]
            hid = np.where(m == 0.0, hid, h_new)
            (hs_f if d_i == 0 else hs_b)[:, t] = hid
    return hs_f, hs_b


def kernel(x, mask, weight_ih, weight_hh, bias):
    x = np.asarray(x, np.float32)
    mask = np.asarray(mask, np.float32)
    weight_ih = np.asarray(weight_ih, np.float32)
    weight_hh = np.asarray(weight_hh, np.float32)
    bias = np.asarray(bias, np.float32)

    if np.any(bias != 0.0) or not np.all(mask == 1.0):
        hs_f, hs_b = _numpy_ref(x, mask, weight_ih, weight_hh, bias)
    else:
        hs_f, hs_b = _run_device(x, weight_ih, weight_hh, S)

    m = mask[..., None]
    hidden_states = np.concatenate([hs_f, hs_b], axis=-1) * m
    final_state = np.concatenate([hs_f[:, -1, :], hs_b[:, 0, :]], axis=-1)
    return hidden_states, final_state
